# revision 30
# baseline (speedup 1.0000x reference)
"""Trainium2 Bass kernel for nn_DHSMoERBFDetector.

Reference math (B=8192, D=768, NC=4, R=128, E=20, H=1024):
    rbf[c,b,r] = exp(coeff[c] * (feats[c,b] - offset[c,r])^2)
    x = [emb | rbf-features]                      # [B, 1280]
    h_e = relu(x @ W1[e] + b1[e]); pred_e = h_e @ W2[e] + b2[e]
    out = concat_e(pred_e)[inv]  with inv = argsort(argsort(component_idx))

Key fact: inv has values < B, so only expert 0's predictions are ever
selected.  The output is exactly  (relu(x @ W1[0] + b1[0]) @ W2[0] + b2[0])[inv].

Strategy: data-parallel over batch, 1024 rows per core on 8 cores.
Each core computes x^T (K-major: contraction dim on partitions) in SBUF --
emb arrives host-transposed, RBF features are generated on-device already
K-major via a K=1 broadcast matmul + ScalarE Square/Exp -- then runs the
expert-0 MLP (K=1280 contraction in 10 chunks of 128; H=1024 as 8 chunks
of M=128; batch as N=512 moving operand) accumulating in PSUM, ReLU fused
with the b1 bias on ScalarE, and an M=1 matmul for the H->1 dot product.
The inverse permutation is an int gather of 8192 floats, done on host.

Matmul precision modes (KERNEL_MODE env var):
  bf16x3 (default): x and W1 split into bf16 value+residual pairs;
      x@W1 = xh@Wh + xh@Wl + xl@Wh -- 3 bf16 matmuls at 1 PE cycle/row
      each vs plain fp32's 4 cycles/row, fp32 PSUM accumulation.
      HW-measured end-to-end error 5.5e-6 relative (fp32 gives 6.6e-7).
  fp32: plain fp32 matmuls (4 cycles/row), bit-nearest the reference.
  f32r1: single-pass f32r matmuls (trn2's 11-bit-mantissa fp32 mode,
      1 cycle/row; ~2e-4 relative error) -- fastest, ~74us/core in the
      cost model vs bf16x3's 136us, if ~2e-4 error is acceptable.
  f32r3/f32rh: f32r-based 3-pass splits (~1e-7); f32rh hit an
      NRT_EXEC_UNIT_UNRECOVERABLE on hardware (mixed-dtype PSUM
      accumulation groups suspected) -- kept for reference only.
"""

import os

import numpy as np

import concourse.bacc as bacc
import concourse.bass as bass
import concourse.mybir as mybir
import concourse.tile as tile
from concourse.bass_utils import run_bass_kernel_spmd

FP32 = mybir.dt.float32
F32R = mybir.dt.float32r
BF16 = mybir.dt.bfloat16
AF = mybir.ActivationFunctionType

B, D, NCOL, R, E, H = 8192, 768, 4, 128, 20, 1024
KTOT = D + NCOL * R          # 1280 contraction dim
NCORES = 8
BL = B // NCORES             # 1024 batch rows per core
KC = KTOT // 128             # 10 k-chunks (0-5 emb, 6-9 rbf)
KC_EMB = D // 128            # 6
HC = H // 128                # 8 hidden chunks
NT = BL // 512               # 2 batch tiles of N=512


def _consts(nc, consts, dram_map):
    sb = {}
    sb["feats"] = consts.tile([1, NCOL * BL], FP32, tag="feats", name="feats_sb")
    sb["noff"] = consts.tile([R, NCOL], FP32, tag="noff", name="noff_sb")
    sb["coef"] = consts.tile([R, NCOL], FP32, tag="coef", name="coef_sb")
    sb["b1"] = consts.tile([128, HC], FP32, tag="b1", name="b1_sb")
    sb["b2"] = consts.tile([1, 1], FP32, tag="b2", name="b2_sb")
    sb["ones"] = consts.tile([1, 128], FP32, tag="ones", name="ones_sb")
    nc.vector.memset(sb["ones"], 1.0)
    for key, src in dram_map.items():
        nc.sync.dma_start(sb[key], src[:, :])
    return sb


def _rbf_psum(nc, pp, sb, c, n):
    """Broadcast feats[c] across partitions into a PSUM tile via K=1 matmul."""
    bc = pp.tile([128, 512], FP32, tag="ps", name=f"bc_{c}_{n}")
    nc.tensor.matmul(
        bc, lhsT=sb["ones"][:, :],
        rhs=sb["feats"][:, c * BL + n * 512 : c * BL + (n + 1) * 512],
        start=True, stop=True,
    )
    return bc


def _build_f32r(nc, tc, dram, pools, three_pass: bool):
    """f32r matmul pipeline; three_pass adds the two residual terms.

    three_pass keeps SBUF under budget by streaming feats tiles, keeping h
    chunks per-n-tile only ([128,512] per m, reused across n), and using a
    plain fp32 second matmul (no h split needed).
    """
    big, consts, tmp, outp, pp = pools
    d = dram
    sb = {}
    sb["noff"] = consts.tile([R, NCOL], FP32, tag="noff", name="noff_sb")
    sb["coef"] = consts.tile([R, NCOL], FP32, tag="coef", name="coef_sb")
    sb["b1"] = consts.tile([128, HC], FP32, tag="b1", name="b1_sb")
    sb["b2"] = consts.tile([1, 1], FP32, tag="b2", name="b2_sb")
    sb["ones"] = consts.tile([1, 128], FP32, tag="ones", name="ones_sb")
    nc.vector.memset(sb["ones"], 1.0)
    for key, src in [("noff", d["noff"]), ("coef", d["coef"]),
                     ("b1", d["b1c"]), ("b2", d["b2c"])]:
        nc.sync.dma_start(sb[key], src[:, :])
    w2r_sb = consts.tile([128, HC], F32R, tag="w2r", name="w2r_sb")
    nc.sync.dma_start(w2r_sb, d["w2r"][:, :])
    w2f_sb = None
    if three_pass:
        w2f_sb = consts.tile([128, HC], FP32, tag="w2f", name="w2f_sb")
        nc.sync.dma_start(w2f_sb, d["w2c"][:, :])

    xr = [big.tile([128, BL], F32R, tag=f"xr{k}", name=f"xr{k}")
          for k in range(KC)]
    wr = [big.tile([128, H], F32R, tag=f"wr{k}", name=f"wr{k}")
          for k in range(KC)]
    if three_pass:
        xl = [big.tile([128, BL], F32R, tag=f"xl{k}", name=f"xl{k}")
              for k in range(KC)]
        wl = [big.tile([128, H], F32R, tag=f"wl{k}", name=f"wl{k}")
              for k in range(KC)]

    # RBF features first: their small feats DMAs must not queue behind the
    # big weight DMAs (the in-order PE's first instruction waits on them).
    for c in range(NCOL):
        for n in range(NT):
            bsl = slice(n * 512, (n + 1) * 512)
            kk = KC_EMB + c
            fe = tmp.tile([1, 512], FP32, tag="fe")
            nc.sync.dma_start(
                fe, d["feats"][:, c * BL + n * 512 : c * BL + (n + 1) * 512])
            bc = pp.tile([128, 512], FP32, tag="ps", name=f"bc_{c}_{n}")
            nc.tensor.matmul(bc, lhsT=sb["ones"][:, :], rhs=fe,
                             start=True, stop=True)
            d2 = tmp.tile([128, 512], FP32, tag="d2")
            nc.scalar.activation(d2, bc, AF.Square,
                                 bias=sb["noff"][:, c : c + 1], scale=1.0)
            if three_pass:
                rb = tmp.tile([128, 512], FP32, tag="rb")
                nc.scalar.activation(rb, d2, AF.Exp,
                                     scale=sb["coef"][:, c : c + 1])
                nc.vector.tensor_copy(xr[kk][:, bsl], rb)   # round to f32r
                nc.vector.tensor_tensor(
                    xl[kk][:, bsl], rb, xr[kk][:, bsl].bitcast(FP32),
                    mybir.AluOpType.subtract,
                )                                           # residual, rounded
            else:
                nc.scalar.activation(xr[kk][:, bsl], d2, AF.Exp,
                                     scale=sb["coef"][:, c : c + 1])

    for k in range(KC):
        ksl = slice(k * 128, (k + 1) * 128)
        nc.sync.dma_start(wr[k][:, :], d["w1r"][ksl, :])
        if three_pass:
            nc.sync.dma_start(wl[k][:, :], d["w1l"][ksl, :])
        if k < KC_EMB:
            nc.sync.dma_start(xr[k][:, :], d["ehr"][ksl, :])
            if three_pass:
                nc.sync.dma_start(xl[k][:, :], d["ehl"][ksl, :])

    # h^T = relu(W1^T x + b1); f32r1 writes relu straight to f32r tiles
    h_dt = FP32 if three_pass else F32R
    h_len = 512 if three_pass else BL
    hs = [big.tile([128, h_len], h_dt, tag=f"h{m}", name=f"h{m}")
          for m in range(HC)]
    for n in range(NT):
        bsl = slice(n * 512, (n + 1) * 512)
        hsl = slice(0, 512) if three_pass else bsl
        # pred accumulates per group so the last group's relu drain overlaps
        # with the earlier groups' pred matmuls
        p2 = pp.tile([1, 512], FP32, tag="ps", name=f"p2_{n}")
        w2 = w2f_sb if three_pass else w2r_sb
        for g in range(2):
            ms = range(4 * g, 4 * g + 4)
            ps = {m: pp.tile([128, 512], FP32, tag="ps", name=f"ps_{n}_{g}_{m}")
                  for m in ms}
            for k in range(KC):
                for m in ms:
                    msl = slice(m * 128, (m + 1) * 128)
                    first, last = k == 0, k == KC - 1
                    if three_pass:
                        nc.tensor.matmul(ps[m], lhsT=wr[k][:, msl],
                                         rhs=xr[k][:, bsl],
                                         start=first, stop=False)
                        nc.tensor.matmul(ps[m], lhsT=wr[k][:, msl],
                                         rhs=xl[k][:, bsl],
                                         start=False, stop=False)
                        nc.tensor.matmul(ps[m], lhsT=wl[k][:, msl],
                                         rhs=xr[k][:, bsl],
                                         start=False, stop=last)
                    else:
                        nc.tensor.matmul(ps[m], lhsT=wr[k][:, msl],
                                         rhs=xr[k][:, bsl],
                                         start=first, stop=last)
            for m in ms:
                nc.scalar.activation(hs[m][:, hsl], ps[m], AF.Relu,
                                     bias=sb["b1"][:, m : m + 1], scale=1.0)
            for m in ms:
                nc.tensor.matmul(p2, lhsT=w2[:, m : m + 1], rhs=hs[m][:, hsl],
                                 start=(m == 0), stop=(m == HC - 1))
        o_sb = outp.tile([1, 512], FP32, tag="o")
        nc.vector.tensor_scalar_add(o_sb, p2, sb["b2"][:1, :1])
        nc.sync.dma_start(d["out"][:, bsl], o_sb)


def _build_f32rh(nc, tc, dram, pools):
    """Hybrid: main term in f32r (11-bit, 1 cycle/row), correction terms in
    bf16.  x@W = xr@wr + xh@wl + xl@wh with xr = f32r(x), xl = bf16(x - xr),
    xh = bf16(x) (same for W).  Error ~5e-7 relative -- fp32-grade -- at the
    same 3-cycles/row PE cost as bf16x3.

    SBUF budget forces: streamed feats tiles, per-n h chunks, bf16 "high"
    planes derived on-device from the f32r planes (zero extra DMA for them).
    """
    big, consts, tmp, outp, pp = pools
    d = dram
    sb = {}
    sb["noff"] = consts.tile([R, NCOL], FP32, tag="noff", name="noff_sb")
    sb["coef"] = consts.tile([R, NCOL], FP32, tag="coef", name="coef_sb")
    sb["b1"] = consts.tile([128, HC], FP32, tag="b1", name="b1_sb")
    sb["b2"] = consts.tile([1, 1], FP32, tag="b2", name="b2_sb")
    sb["ones"] = consts.tile([1, 128], FP32, tag="ones", name="ones_sb")
    nc.vector.memset(sb["ones"], 1.0)
    for key, src in [("noff", d["noff"]), ("coef", d["coef"]),
                     ("b1", d["b1c"]), ("b2", d["b2c"])]:
        nc.sync.dma_start(sb[key], src[:, :])
    w2f_sb = consts.tile([128, HC], FP32, tag="w2f", name="w2f_sb")
    nc.sync.dma_start(w2f_sb, d["w2c"][:, :])

    xr = [big.tile([128, BL], F32R, tag=f"xr{k}", name=f"xr{k}")
          for k in range(KC)]
    xh = [big.tile([128, BL], BF16, tag=f"xh{k}", name=f"xh{k}")
          for k in range(KC)]
    xl = [big.tile([128, BL], BF16, tag=f"xl{k}", name=f"xl{k}")
          for k in range(KC)]
    wr = [big.tile([128, H], F32R, tag=f"wr{k}", name=f"wr{k}")
          for k in range(KC)]
    wh = [big.tile([128, H], BF16, tag=f"wh{k}", name=f"wh{k}")
          for k in range(KC)]
    wl = [big.tile([128, H], BF16, tag=f"wl{k}", name=f"wl{k}")
          for k in range(KC)]

    # RBF features first (small feats DMAs must beat the big DMAs into the
    # queues; the in-order PE's first instruction waits on them)
    for c in range(NCOL):
        for n in range(NT):
            bsl = slice(n * 512, (n + 1) * 512)
            kk = KC_EMB + c
            fe = tmp.tile([1, 512], FP32, tag="fe")
            nc.sync.dma_start(
                fe, d["feats"][:, c * BL + n * 512 : c * BL + (n + 1) * 512])
            bc = pp.tile([128, 512], FP32, tag="ps", name=f"bc_{c}_{n}")
            nc.tensor.matmul(bc, lhsT=sb["ones"][:, :], rhs=fe,
                             start=True, stop=True)
            d2 = tmp.tile([128, 512], FP32, tag="d2")
            nc.scalar.activation(d2, bc, AF.Square,
                                 bias=sb["noff"][:, c : c + 1], scale=1.0)
            rb = tmp.tile([128, 512], FP32, tag="rb")
            nc.scalar.activation(rb, d2, AF.Exp,
                                 scale=sb["coef"][:, c : c + 1])
            nc.vector.tensor_copy(xr[kk][:, bsl], rb)       # round to f32r
            nc.vector.tensor_copy(xh[kk][:, bsl], rb)       # round to bf16
            back = tmp.tile([128, 512], FP32, tag="back")
            nc.vector.tensor_sub(back, rb, xr[kk][:, bsl].bitcast(FP32))
            nc.vector.tensor_copy(xl[kk][:, bsl], back)     # residual -> bf16

    # big DMAs (k-ascending so the first k-sweep streams) + derived bf16
    # "high" planes (DVE casts of the f32r planes; the 2^-12 difference vs
    # bf16(original) only enters the ~2^-13-scale correction terms)
    for k in range(KC):
        ksl = slice(k * 128, (k + 1) * 128)
        nc.sync.dma_start(wr[k][:, :], d["w1r"][ksl, :])
        nc.sync.dma_start(wl[k][:, :], d["w1lb"][ksl, :])
        if k < KC_EMB:
            nc.sync.dma_start(xr[k][:, :], d["ehr"][ksl, :])
            nc.sync.dma_start(xl[k][:, :], d["ehlb"][ksl, :])
            nc.vector.tensor_copy(xh[k][:, :], xr[k].bitcast(FP32))
        nc.vector.tensor_copy(wh[k][:, :], wr[k].bitcast(FP32))

    hs = [big.tile([128, 512], FP32, tag=f"h{m}", name=f"h{m}")
          for m in range(HC)]
    for n in range(NT):
        bsl = slice(n * 512, (n + 1) * 512)
        hsl = slice(0, 512)
        p2 = pp.tile([1, 512], FP32, tag="ps", name=f"p2_{n}")
        for g in range(2):
            ms = range(4 * g, 4 * g + 4)
            ps = {m: pp.tile([128, 512], FP32, tag="ps", name=f"ps_{n}_{g}_{m}")
                  for m in ms}
            for k in range(KC):
                for m in ms:
                    msl = slice(m * 128, (m + 1) * 128)
                    nc.tensor.matmul(ps[m], lhsT=wr[k][:, msl],
                                     rhs=xr[k][:, bsl],
                                     start=(k == 0), stop=False)
                    nc.tensor.matmul(ps[m], lhsT=wh[k][:, msl],
                                     rhs=xl[k][:, bsl],
                                     start=False, stop=False)
                    nc.tensor.matmul(ps[m], lhsT=wl[k][:, msl],
                                     rhs=xh[k][:, bsl],
                                     start=False, stop=(k == KC - 1))
            for m in ms:
                nc.scalar.activation(hs[m][:, hsl], ps[m], AF.Relu,
                                     bias=sb["b1"][:, m : m + 1], scale=1.0)
            for m in ms:
                nc.tensor.matmul(p2, lhsT=w2f_sb[:, m : m + 1],
                                 rhs=hs[m][:, hsl],
                                 start=(m == 0), stop=(m == HC - 1))
        o_sb = outp.tile([1, 512], FP32, tag="o")
        nc.vector.tensor_scalar_add(o_sb, p2, sb["b2"][:1, :1])
        nc.sync.dma_start(d["out"][:, bsl], o_sb)


def _build_fp32(nc, tc, dram, pools):
    big, consts, tmp, outp, pp = pools
    d = dram
    sb = _consts(nc, consts, dict(
        feats=d["feats"], noff=d["noff"], coef=d["coef"],
        b1=d["b1c"], b2=d["b2c"],
    ))
    w2_sb = consts.tile([128, HC], FP32, tag="w2")
    nc.sync.dma_start(w2_sb, d["w2c"][:, :])

    xt = [big.tile([128, BL], FP32, tag=f"xt{k}", name=f"xt{k}")
          for k in range(KC)]
    w1s = [big.tile([128, H], FP32, tag=f"w1_{k}", name=f"w1_{k}")
           for k in range(KC)]
    hs = [big.tile([128, BL], FP32, tag=f"h{m}", name=f"h{m}")
          for m in range(HC)]

    for k in range(KC):
        nc.sync.dma_start(w1s[k][:, :], d["w1"][k * 128 : (k + 1) * 128, :])
        if k < KC_EMB:
            nc.sync.dma_start(xt[k][:, :], d["embT"][k * 128 : (k + 1) * 128, :])

    for c in range(NCOL):
        for n in range(NT):
            bsl = slice(n * 512, (n + 1) * 512)
            bc = _rbf_psum(nc, pp, sb, c, n)
            d2 = tmp.tile([128, 512], FP32, tag="d2")
            nc.scalar.activation(d2, bc, AF.Square,
                                 bias=sb["noff"][:, c : c + 1], scale=1.0)
            nc.scalar.activation(xt[KC_EMB + c][:, bsl], d2, AF.Exp,
                                 scale=sb["coef"][:, c : c + 1])

    for n in range(NT):
        bsl = slice(n * 512, (n + 1) * 512)
        for g in range(2):
            ms = range(4 * g, 4 * g + 4)
            ps = {m: pp.tile([128, 512], FP32, tag="ps", name=f"ps_{n}_{g}_{m}")
                  for m in ms}
            for k in range(KC):
                for m in ms:
                    nc.tensor.matmul(
                        ps[m], lhsT=w1s[k][:, m * 128 : (m + 1) * 128],
                        rhs=xt[k][:, bsl],
                        start=(k == 0), stop=(k == KC - 1),
                    )
            for m in ms:
                nc.scalar.activation(hs[m][:, bsl], ps[m], AF.Relu,
                                     bias=sb["b1"][:, m : m + 1], scale=1.0)

    for n in range(NT):
        bsl = slice(n * 512, (n + 1) * 512)
        p2 = pp.tile([1, 512], FP32, tag="ps", name=f"p2_{n}")
        for m in range(HC):
            nc.tensor.matmul(p2, lhsT=w2_sb[:, m : m + 1], rhs=hs[m][:, bsl],
                             start=(m == 0), stop=(m == HC - 1))
        o_sb = outp.tile([1, 512], FP32, tag="o")
        nc.vector.tensor_scalar_add(o_sb, p2, sb["b2"][:1, :1])
        nc.sync.dma_start(d["out"][:, bsl], o_sb)


def _build_bf16x3(nc, tc, dram, pools):
    big, consts, tmp, outp, pp = pools
    d = dram
    sb = _consts(nc, consts, dict(
        feats=d["feats"], noff=d["noff"], coef=d["coef"],
        b1=d["b1c"], b2=d["b2c"],
    ))
    w2_sb = consts.tile([128, HC], FP32, tag="w2")
    nc.sync.dma_start(w2_sb, d["w2c"][:, :])

    xh = [big.tile([128, BL], BF16, tag=f"xh{k}", name=f"xh{k}")
          for k in range(KC)]
    xl = [big.tile([128, BL], BF16, tag=f"xl{k}", name=f"xl{k}")
          for k in range(KC)]
    wh = [big.tile([128, H], BF16, tag=f"wh{k}", name=f"wh{k}")
          for k in range(KC)]
    wl = [big.tile([128, H], BF16, tag=f"wl{k}", name=f"wl{k}")
          for k in range(KC)]
    hs = [big.tile([128, BL], FP32, tag=f"h{m}", name=f"h{m}")
          for m in range(HC)]

    for k in range(KC):
        ksl = slice(k * 128, (k + 1) * 128)
        nc.sync.dma_start(wh[k][:, :], d["w1h"][ksl, :])
        nc.sync.dma_start(wl[k][:, :], d["w1l"][ksl, :])
        if k < KC_EMB:
            nc.sync.dma_start(xh[k][:, :], d["ehT"][ksl, :])
            nc.sync.dma_start(xl[k][:, :], d["elT"][ksl, :])

    for c in range(NCOL):
        for n in range(NT):
            bsl = slice(n * 512, (n + 1) * 512)
            kk = KC_EMB + c
            bc = _rbf_psum(nc, pp, sb, c, n)
            d2 = tmp.tile([128, 512], FP32, tag="d2")
            nc.scalar.activation(d2, bc, AF.Square,
                                 bias=sb["noff"][:, c : c + 1], scale=1.0)
            rb = tmp.tile([128, 512], FP32, tag="rb")
            nc.scalar.activation(rb, d2, AF.Exp,
                                 scale=sb["coef"][:, c : c + 1])
            nc.vector.tensor_copy(xh[kk][:, bsl], rb)      # round to bf16
            back = tmp.tile([128, 512], FP32, tag="back")
            nc.vector.tensor_copy(back, xh[kk][:, bsl])    # widen
            nc.vector.tensor_sub(back, rb, back)           # residual
            nc.vector.tensor_copy(xl[kk][:, bsl], back)

    for n in range(NT):
        bsl = slice(n * 512, (n + 1) * 512)
        p2 = pp.tile([1, 512], FP32, tag="ps", name=f"p2_{n}")
        for g in range(2):
            ms = range(4 * g, 4 * g + 4)
            ps = {m: pp.tile([128, 512], FP32, tag="ps", name=f"ps_{n}_{g}_{m}")
                  for m in ms}
            for k in range(KC):
                for m in ms:
                    msl = slice(m * 128, (m + 1) * 128)
                    nc.tensor.matmul(ps[m], lhsT=wh[k][:, msl],
                                     rhs=xh[k][:, bsl],
                                     start=(k == 0), stop=False)
                    nc.tensor.matmul(ps[m], lhsT=wh[k][:, msl],
                                     rhs=xl[k][:, bsl],
                                     start=False, stop=False)
                    nc.tensor.matmul(ps[m], lhsT=wl[k][:, msl],
                                     rhs=xh[k][:, bsl],
                                     start=False, stop=(k == KC - 1))
            for m in ms:
                nc.scalar.activation(hs[m][:, bsl], ps[m], AF.Relu,
                                     bias=sb["b1"][:, m : m + 1], scale=1.0)
            for m in ms:
                nc.tensor.matmul(p2, lhsT=w2_sb[:, m : m + 1],
                                 rhs=hs[m][:, bsl],
                                 start=(m == 0), stop=(m == HC - 1))
        o_sb = outp.tile([1, 512], FP32, tag="o")
        nc.vector.tensor_scalar_add(o_sb, p2, sb["b2"][:1, :1])
        nc.sync.dma_start(d["out"][:, bsl], o_sb)


def _build_nc(mode: str) -> bass.Bass:
    # Bacc (not raw Bass): its finalize() runs move_matmul_waits_to_ldweights
    # + generate_event_semaphores, which split semaphore waits that exceed
    # the per-instruction hardware limit (walrus otherwise fails codegen).
    nc = bacc.Bacc()

    d = {}
    d["feats"] = nc.dram_tensor("feats", [1, NCOL * BL], FP32,
                                kind="ExternalInput")
    d["b1c"] = nc.dram_tensor("b1c", [128, HC], FP32, kind="ExternalInput")
    d["w2c"] = nc.dram_tensor("w2c", [128, HC], FP32, kind="ExternalInput")
    d["b2c"] = nc.dram_tensor("b2c", [1, 1], FP32, kind="ExternalInput")
    d["noff"] = nc.dram_tensor("noff", [R, NCOL], FP32, kind="ExternalInput")
    d["coef"] = nc.dram_tensor("coef", [R, NCOL], FP32, kind="ExternalInput")
    d["out"] = nc.dram_tensor("out", [1, BL], FP32, kind="ExternalOutput")

    if mode == "fp32":
        d["embT"] = nc.dram_tensor("embT", [D, BL], FP32, kind="ExternalInput")
        d["w1"] = nc.dram_tensor("w1", [KTOT, H], FP32, kind="ExternalInput")
    elif mode == "bf16x3":
        for n2 in ("ehT", "elT"):
            d[n2] = nc.dram_tensor(n2, [D, BL], BF16, kind="ExternalInput")
        for n2 in ("w1h", "w1l"):
            d[n2] = nc.dram_tensor(n2, [KTOT, H], BF16, kind="ExternalInput")
    elif mode in ("f32r1", "f32r3"):
        d["ehr"] = nc.dram_tensor("ehr", [D, BL], F32R, kind="ExternalInput")
        d["w1r"] = nc.dram_tensor("w1r", [KTOT, H], F32R, kind="ExternalInput")
        d["w2r"] = nc.dram_tensor("w2r", [128, HC], F32R, kind="ExternalInput")
        if mode == "f32r3":
            d["ehl"] = nc.dram_tensor("ehl", [D, BL], F32R,
                                      kind="ExternalInput")
            d["w1l"] = nc.dram_tensor("w1l", [KTOT, H], F32R,
                                      kind="ExternalInput")
            d["w2l"] = nc.dram_tensor("w2l", [128, HC], F32R,
                                      kind="ExternalInput")
    elif mode == "f32rh":
        d["ehr"] = nc.dram_tensor("ehr", [D, BL], F32R, kind="ExternalInput")
        d["ehlb"] = nc.dram_tensor("ehlb", [D, BL], BF16, kind="ExternalInput")
        d["w1r"] = nc.dram_tensor("w1r", [KTOT, H], F32R, kind="ExternalInput")
        d["w1lb"] = nc.dram_tensor("w1lb", [KTOT, H], BF16,
                                   kind="ExternalInput")
    else:
        raise ValueError(mode)

    with tile.TileContext(nc) as tc:
        with (
            tc.tile_pool(name="big", bufs=1) as big,
            tc.tile_pool(name="consts", bufs=1) as consts,
            tc.tile_pool(name="tmp", bufs=3) as tmp,
            tc.tile_pool(name="outp", bufs=2) as outp,
            tc.tile_pool(name="psum", bufs=8, space="PSUM") as pp,
        ):
            pools = (big, consts, tmp, outp, pp)
            if mode == "fp32":
                _build_fp32(nc, tc, d, pools)
            elif mode == "bf16x3":
                _build_bf16x3(nc, tc, d, pools)
            elif mode == "f32rh":
                _build_f32rh(nc, tc, d, pools)
            else:
                _build_f32r(nc, tc, d, pools, three_pass=(mode == "f32r3"))

    # run Bacc's compile pipeline (wait splitting, register allocation);
    # run_bass_via_pjrt serializes nc.m as-is and never finalizes.
    nc.finalize()
    return nc


def _bf16_pair(a: np.ndarray):
    """Split fp32 array into (hi, lo) bf16 arrays with hi+lo ~ a."""
    import ml_dtypes

    hi = a.astype(ml_dtypes.bfloat16)
    lo = (a - hi.astype(np.float32)).astype(ml_dtypes.bfloat16)
    return hi, lo


def _round_f32r(a: np.ndarray) -> np.ndarray:
    """Round fp32 to f32r (11-bit mantissa, round-half-up at bit 12) --
    bit-exact with the hardware's cast (verified against gpsimd cast-DMA)."""
    v = np.ascontiguousarray(a, dtype=np.float32).view(np.uint32)
    r = (((v.astype(np.uint64) + (1 << 11)) >> 12) << 12).astype(np.uint32)
    return r.view(np.float32)


def _f32r_pair(a: np.ndarray):
    hi = _round_f32r(a)
    lo = _round_f32r(a - hi)
    return hi, lo


_NC_CACHE: dict = {}


def kernel(emb, feats, rbf_offset, rbf_coeff, W1, b1, W2, b2, component_idx):
    mode = os.environ.get("KERNEL_MODE", "f32r3")
    emb = np.ascontiguousarray(emb, dtype=np.float32)
    feats = np.ascontiguousarray(feats, dtype=np.float32)
    rbf_offset = np.asarray(rbf_offset, dtype=np.float32)
    rbf_coeff = np.asarray(rbf_coeff, dtype=np.float32)
    W1 = np.asarray(W1, dtype=np.float32)
    b1 = np.asarray(b1, dtype=np.float32)
    W2 = np.asarray(W2, dtype=np.float32)
    b2 = np.asarray(b2, dtype=np.float32)
    component_idx = np.asarray(component_idx)

    # shared (expert-0 only) tensors
    w1_full = np.ascontiguousarray(W1[0])                        # [1280, 1024]
    w2c = np.ascontiguousarray(W2[0, :, 0].reshape(HC, 128).T)   # [128, 8]
    shared = dict(
        b1c=np.ascontiguousarray(b1[0].reshape(HC, 128).T),      # [128, 8]
        w2c=w2c,
        b2c=b2[0].reshape(1, 1),
        noff=np.ascontiguousarray(-rbf_offset.T),                # [128, 4]
        coef=np.ascontiguousarray(
            np.broadcast_to(rbf_coeff[None, :], (R, NCOL))),     # [128, 4]
    )
    if mode == "fp32":
        shared["w1"] = w1_full
    elif mode == "bf16x3":
        shared["w1h"], shared["w1l"] = _bf16_pair(w1_full)
    elif mode == "f32rh":
        import ml_dtypes

        shared["w1r"] = _round_f32r(w1_full)
        shared["w1lb"] = (w1_full - shared["w1r"]).astype(ml_dtypes.bfloat16)
    else:
        shared["w1r"], w1l = _f32r_pair(w1_full)
        w2r, w2l = _f32r_pair(w2c)
        shared["w2r"] = w2r
        if mode == "f32r3":
            shared["w1l"] = w1l
            shared["w2l"] = w2l

    in_maps = []
    for i in range(NCORES):
        s = slice(i * BL, (i + 1) * BL)
        m = dict(
            feats=np.ascontiguousarray(feats[:, s]).reshape(1, NCOL * BL),
            **shared,
        )
        embT = np.ascontiguousarray(emb[s].T)                    # [768, 1024]
        if mode == "fp32":
            m["embT"] = embT
        elif mode == "bf16x3":
            m["ehT"], m["elT"] = _bf16_pair(embT)
        elif mode == "f32rh":
            import ml_dtypes

            m["ehr"] = _round_f32r(embT)
            m["ehlb"] = (embT - m["ehr"]).astype(ml_dtypes.bfloat16)
        else:
            m["ehr"], ehl = _f32r_pair(embT)
            if mode == "f32r3":
                m["ehl"] = ehl
        in_maps.append(m)

    if mode not in _NC_CACHE:
        _NC_CACHE[mode] = _build_nc(mode)

    res = run_bass_kernel_spmd(_NC_CACHE[mode], in_maps, list(range(NCORES)))

    pred = np.concatenate(
        [res.results[i]["out"].reshape(BL) for i in range(NCORES)]
    )                                                            # [8192]

    order = np.argsort(component_idx, kind="stable")
    inv = np.argsort(order, kind="stable")
    return pred[inv].reshape(B, 1).astype(np.float32)
